# revision 3
# baseline (speedup 1.0000x reference)
"""Trainium2 Bass kernel for nn_CategoricalProjection (C51 distributional-RL
categorical projection / histogram binning).

Math: out[b, j] = sum_a p[b,a] * hat(pos[b,a] - j),  hat(x) = max(0, 1-|x|),
pos = clip(alpha_b + beta_b * a, 0, 50), alpha = 2.5*reward + 25 - 24.75*nd,
beta = 0.99*nd.

Implementation: out[j] = d2/dj2 of R, R[j] = sum_a p_a * relu(j - pos_a)
             = j * PP[c_j] - PV[c_j],
PP/PV = prefix sums of p and p*pos over atoms, c_j = #atoms with pos_raw < j.
The per-row gather PP[c_j] is realised as the inverse scatter: value PP[c]
goes to dst column first_j(c) (affine in c), collisions redirected to trash
columns, then a running-max fill (PP/PV are non-decreasing) recovers the
gathered sequence. Scatter is GPSIMD local_scatter (per-partition, int16);
everything else is fused custom DVE ops.

Sharding: pure data-parallel over the batch across 8 NeuronCores.
"""
import numpy as np

import concourse.bacc as bacc
import concourse.tile as tile
from concourse import mybir
from concourse.bass_utils import run_bass_kernel_spmd

# ---- problem constants (hardcoded per harness contract) ----
BS = 524288
A = 51
N_CORES = 8
ROWS = BS // N_CORES          # 65536 rows per core
P = 128                       # partitions
G = 8                         # tiles per macro-tile
TILE_ROWS = P                 # rows per tile
MACRO_ROWS = P * G            # 1024
M = ROWS // MACRO_ROWS        # 64 macros

F = 2                         # dst window base
CS_P = 31744.0                # PP fixed-point scale
CS_V = 496.0                  # PV fixed-point scale
RATIO = CS_P / CS_V           # 64.0
NE = 128                      # local_scatter dst elements
W53 = 53
VS = 56                       # stride per group in the v-col tile

f32 = mybir.dt.float32
i16 = mybir.dt.int16


# ---------------------------------------------------------------------------
# custom DVE ops
# ---------------------------------------------------------------------------
_OPS = {}


def _register_ops():
    if _OPS:
        return _OPS
    from concourse import dve_ops as dvo
    from concourse.dve_spec import (
        Spec, Src0, Src1, C0, C1, C2, scan, AluOp, select, Idx, Zero, maxx,
        minn, lower, eq,
    )
    from concourse.dve_ops import has_src1
    from concourse.dve_table_gen import DveOpSpec

    def reg(name, spec, subdim=False):
        for existing in dvo.OPS:
            if existing.name == name:
                _OPS[name] = existing
                return
        row = dvo._CUSTOM_DVE_ROW_BASE + len(dvo.OPS)
        assert row < 0x20, "custom DVE row overflow"
        dvo._SUB_OPCODE_FOR_NAME[name] = row
        shas = {}
        for ver in ("v3", "v4"):
            s = DveOpSpec(name=name, opcode=row, uops=lower(spec, ver=ver),
                          rd1_en=has_src1(spec))
            shas[ver] = s.sha(ver)
        op = dvo.DveOp(name, spec, subdim, uops_sha=shas)
        dvo.OPS.append(op)
        dvo.CUSTOM_DVE_SPECS[name] = spec
        _OPS[name] = op

    # PP-scan: out_i16[k] = trunc(cumsum(p * CS_P))
    def _pp_ref(in0, in1, s0, s1, imm2):
        r = np.cumsum(in0.astype(np.float32) * np.float32(imm2), axis=-1,
                      dtype=np.float32)
        return np.clip(np.rint(r), -32768, 32767)

    reg("CP_PPSCAN", Spec(body=scan(AluOp.ADD, Src0 * C2), reference=_pp_ref))

    # PV-scan: out_i16[k] = trunc(cumsum(p * clip(c0 + c1*a, 0, imm2)))
    # in1 = const row of atom indices a = 0..50 (fp32)
    def _pv_ref(in0, in1, s0, s1, imm2):
        pos = np.clip(s0 + s1 * in1.astype(np.float32), 0.0, np.float32(imm2))
        r = np.cumsum(in0.astype(np.float32) * pos, axis=-1, dtype=np.float32)
        return np.clip(np.rint(r), -32768, 32767)

    reg("CP_PVSCAN",
        Spec(body=scan(AluOp.ADD, Src0 * minn(maxx(C0 + C1 * Src1, Zero), C2)),
             reference=_pv_ref))

    # v-col: out_i16 = max(min(c*beta + c0, imm2), lorow)
    # in0 = const row c = 0..52; in1 = const row [LO]*52 + [20000]
    def _vc_ref(in0, in1, s0, s1, imm2):
        v = in0.astype(np.float32) * s1 + s0
        v = np.maximum(np.minimum(v, np.float32(imm2)), in1.astype(np.float32))
        return np.clip(np.rint(v), -32768, 32767)

    reg("CP_VCOL", Spec(body=maxx(minn(Src0 * C1 + C0, C2), Src1),
                        reference=_vc_ref))

    # redirect: out = (in0 == in1) ? Idx + c0 : in0   (int16 streams)
    def _rd_ref(in0, in1, s0, s1, imm2):
        idx = np.arange(in0.shape[-1], dtype=np.float32)
        return np.where(in0 == in1, idx + s0, in0.astype(np.float32))

    reg("CP_REDIR", Spec(body=select(eq(Src0, Src1), Idx + C0, Src0),
                         reference=_rd_ref))

    # R-op: gP = scanmax(in0), gV = scanmax(in1); R = gP*Idx - gV*c0
    def _r_ref(in0, in1, s0, s1, imm2):
        gP = np.maximum.accumulate(in0.astype(np.float32), axis=-1)
        gV = np.maximum.accumulate(in1.astype(np.float32), axis=-1)
        idx = np.arange(in0.shape[-1], dtype=np.float32)
        return gP * idx - gV * s0

    reg("CP_ROP", Spec(body=scan(AluOp.MAX, Src0) * Idx
                       - scan(AluOp.MAX, Src1) * C0, reference=_r_ref))

    # final: out = (in0 - in1) * c0
    def _fin_ref(in0, in1, s0, s1, imm2):
        return (in0.astype(np.float32) - in1.astype(np.float32)) * s0

    reg("CP_FINAL", Spec(body=(Src0 - Src1) * C0, reference=_fin_ref))
    return _OPS


# ---------------------------------------------------------------------------
# program builder
# ---------------------------------------------------------------------------
def _build(n_macros=M):
    ops = _register_ops()
    nc = bacc.Bacc()
    nrows = n_macros * MACRO_ROWS

    reward_in = nc.dram_tensor("reward", [nrows, 1], f32, kind="ExternalInput")
    probs_in = nc.dram_tensor("probs", [nrows, A], f32, kind="ExternalInput")
    nd_in = nc.dram_tensor("not_done", [nrows, 1], f32, kind="ExternalInput")
    crow_in = nc.dram_tensor("crow", [P, W53], f32, kind="ExternalInput")
    lorow_in = nc.dram_tensor("lorow", [P, W53], f32, kind="ExternalInput")
    out_t = nc.dram_tensor("out", [nrows, A], f32, kind="ExternalOutput")

    # row(m, p, g) = m*1024 + p*8 + g
    pr = probs_in[:].rearrange("(m p g) c -> m p (g c)", m=n_macros, p=P, g=G)
    rr = reward_in[:].rearrange("(m p g) c -> m p (g c)", m=n_macros, p=P, g=G)
    ndr = nd_in[:].rearrange("(m p g) c -> m p (g c)", m=n_macros, p=P, g=G)
    outr = out_t[:].rearrange("(m p g) c -> m p (g c)", m=n_macros, p=P, g=G)

    AluOp = mybir.AluOpType

    with tile.TileContext(nc) as tc:
        with tc.tile_pool(name="consts", bufs=1) as cpool, \
             tc.tile_pool(name="work", bufs=2) as pool, \
             tc.tile_pool(name="dsts", bufs=2) as dpool:
            crow = cpool.tile([P, W53], f32)
            nc.sync.dma_start(out=crow[:], in_=crow_in[:])
            lorow = cpool.tile([P, W53], f32)
            nc.sync.dma_start(out=lorow[:], in_=lorow_in[:])

            for mi in range(n_macros):
                ptile = pool.tile([P, G * A], f32, tag="ptile")
                nc.sync.dma_start(out=ptile[:], in_=pr[mi])
                rt = pool.tile([P, G], f32, tag="rt")
                nc.sync.dma_start(out=rt[:], in_=rr[mi])
                ndt = pool.tile([P, G], f32, tag="ndt")
                nc.sync.dma_start(out=ndt[:], in_=ndr[mi])

                # ---- scalar block (per-row values, one column per tile g)
                alpha = pool.tile([P, G], f32, tag="alpha")
                # t1 = 2.5*r + 25
                nc.vector.tensor_scalar(alpha[:], rt[:], 2.5, 25.0,
                                        AluOp.mult, AluOp.add)
                # alpha = -24.75*nd + t1
                nc.vector.scalar_tensor_tensor(alpha[:], ndt[:], -24.75,
                                               alpha[:], AluOp.mult, AluOp.add)
                beta = pool.tile([P, G], f32, tag="beta")
                nc.vector.tensor_scalar(beta[:], ndt[:], 0.99, None, AluOp.mult)
                c0v = pool.tile([P, G], f32, tag="c0v")
                # c0v = alpha - beta + (F + 2)
                nc.vector.scalar_tensor_tensor(c0v[:], beta[:], -1.0, alpha[:],
                                               AluOp.mult, AluOp.add)
                nc.vector.tensor_scalar(c0v[:], c0v[:], float(F + 2) - 0.5,
                                        None, AluOp.add)
                apv = pool.tile([P, G], f32, tag="apv")
                nc.vector.tensor_scalar(apv[:], alpha[:], CS_V, None, AluOp.mult)
                bpv = pool.tile([P, G], f32, tag="bpv")
                nc.vector.tensor_scalar(bpv[:], beta[:], CS_V, None, AluOp.mult)

                # ---- per-macro tiles
                pps = pool.tile([P, G * 52], i16, tag="pps")
                pvs = pool.tile([P, G * 52], i16, tag="pvs")
                vcols = pool.tile([P, G * VS], i16, tag="vcols")
                cols = pool.tile([P, G * 52], i16, tag="cols")
                rtile = pool.tile([P, G * W53], f32, tag="rtile")
                dstp = dpool.tile([P, G * NE], i16, tag="dstp")
                dstv = dpool.tile([P, G * NE], i16, tag="dstv")

                pps_r = pps[:].rearrange("p (g c) -> p g c", g=G)
                pvs_r = pvs[:].rearrange("p (g c) -> p g c", g=G)
                rtile_r = rtile[:].rearrange("p (g c) -> p g c", g=G)

                # zero the per-group col-0 of the prefix tiles and R tiles
                nc.vector.memset(pps_r[:, :, 0:1], 0)
                nc.vector.memset(pvs_r[:, :, 0:1], 0)
                nc.vector.memset(rtile_r[:, :, 0:1], 0.0)

                for g in range(G):
                    psl = ptile[:, g * A:(g + 1) * A]
                    # prefix sums (int16 fixed point), exclusive via col 0 = 0
                    nc.vector._custom_dve(
                        ops["CP_PPSCAN"],
                        out=pps[:, g * 52 + 1:(g + 1) * 52],
                        in0=psl, imm2=CS_P)
                    nc.vector._custom_dve(
                        ops["CP_PVSCAN"],
                        out=pvs[:, g * 52 + 1:(g + 1) * 52],
                        in0=psl, in1=crow[:, 0:A],
                        s0=apv[:, g:g + 1], s1=bpv[:, g:g + 1],
                        imm2=50.0 * CS_V)
                    # scatter columns
                    nc.vector._custom_dve(
                        ops["CP_VCOL"],
                        out=vcols[:, g * VS:g * VS + W53],
                        in0=crow[:], in1=lorow[:],
                        s0=c0v[:, g:g + 1], s1=beta[:, g:g + 1],
                        imm2=float(F) + 52.4)
                    nc.vector._custom_dve(
                        ops["CP_REDIR"],
                        out=cols[:, g * 52:(g + 1) * 52],
                        in0=vcols[:, g * VS:g * VS + 52],
                        in1=vcols[:, g * VS + 1:g * VS + W53],
                        s0=60.0)
                    # per-partition scatters
                    nc.gpsimd.local_scatter(
                        dstp[:, g * NE:(g + 1) * NE],
                        pps[:, g * 52:(g + 1) * 52],
                        cols[:, g * 52:(g + 1) * 52],
                        channels=P, num_elems=NE, num_idxs=52)
                    nc.gpsimd.local_scatter(
                        dstv[:, g * NE:(g + 1) * NE],
                        pvs[:, g * 52:(g + 1) * 52],
                        cols[:, g * 52:(g + 1) * 52],
                        channels=P, num_elems=NE, num_idxs=52)
                    # R[j] = scanmax-fill + j*gP - RATIO*gV  (PP units)
                    nc.vector._custom_dve(
                        ops["CP_ROP"],
                        out=rtile[:, g * W53 + 1:(g + 1) * W53],
                        in0=dstp[:, g * NE + F + 1:g * NE + F + 53],
                        in1=dstv[:, g * NE + F + 1:g * NE + F + 53],
                        s0=RATIO)

                # d = R[k+1] - R[k]; out = (d[k+1] - d[k]) / CS_P
                dtile = pool.tile([P, G * 52], f32, tag="dtile")
                dtile_r = dtile[:].rearrange("p (g c) -> p g c", g=G)
                nc.vector.tensor_tensor(
                    dtile_r[:], rtile_r[:, :, 1:53], rtile_r[:, :, 0:52],
                    AluOp.subtract)
                otile = pool.tile([P, G * A], f32, tag="otile")
                otile_r = otile[:].rearrange("p (g c) -> p g c", g=G)
                nc.vector._custom_dve(
                    ops["CP_FINAL"],
                    out=otile_r[:],
                    in0=dtile_r[:, :, 1:52], in1=dtile_r[:, :, 0:51],
                    s0=1.0 / CS_P)
                nc.scalar.dma_start(out=outr[mi], in_=otile[:])

    nc.compile()
    return nc


_CONSTS = None


def _const_inputs():
    global _CONSTS
    if _CONSTS is None:
        crow = np.tile(np.arange(W53, dtype=np.float32), (P, 1))
        lorow = np.tile(
            np.array([float(F) + 1.4] * 52 + [20000.0], dtype=np.float32),
            (P, 1))
        _CONSTS = {"crow": crow, "lorow": lorow}
    return _CONSTS


_PROGRAM = None


def kernel(reward, probs, not_done):
    global _PROGRAM
    reward = np.ascontiguousarray(np.asarray(reward, dtype=np.float32))
    probs = np.ascontiguousarray(np.asarray(probs, dtype=np.float32))
    not_done = np.ascontiguousarray(np.asarray(not_done, dtype=np.float32))
    assert reward.shape == (BS, 1) and probs.shape == (BS, A)

    if _PROGRAM is None:
        _PROGRAM = _build(M)
    consts = _const_inputs()

    in_maps = []
    for c in range(N_CORES):
        sl = slice(c * ROWS, (c + 1) * ROWS)
        in_maps.append({
            "reward": reward[sl],
            "probs": probs[sl],
            "not_done": not_done[sl],
            **consts,
        })
    res = run_bass_kernel_spmd(_PROGRAM, in_maps, list(range(N_CORES)))
    out = np.empty((BS, A), dtype=np.float32)
    for c in range(N_CORES):
        out[c * ROWS:(c + 1) * ROWS] = res.results[c]["out"]
    return out


# revision 4
# speedup vs baseline: 1.1239x; 1.1239x over previous
"""Trainium2 Bass kernel for nn_CategoricalProjection (C51 distributional-RL
categorical projection / histogram binning).

Math: out[b, j] = sum_a p[b,a] * hat(pos[b,a] - j),  hat(x) = max(0, 1-|x|),
pos = clip(alpha_b + beta_b * a, 0, 50), alpha = 2.5*reward + 25 - 24.75*nd,
beta = 0.99*nd.

Implementation: out[j] = d2/dj2 of R, R[j] = sum_a p_a * relu(j - pos_a)
             = j * PP[c_j] - PV[c_j],
PP/PV = prefix sums of p and p*pos over atoms, c_j = #atoms with pos_raw < j.
The per-row gather PP[c_j] is realised as the inverse scatter: value PP[c]
goes to dst column first_j(c) (affine in c), collisions redirected to trash
columns, then a running-max fill (PP/PV are non-decreasing) recovers the
gathered sequence. Scatter is GPSIMD local_scatter (per-partition, int16);
everything else is fused custom DVE ops.

Sharding: pure data-parallel over the batch across 8 NeuronCores.
"""
import numpy as np

import concourse.bacc as bacc
import concourse.tile as tile
from concourse import mybir
from concourse.bass_utils import run_bass_kernel_spmd

# ---- problem constants (hardcoded per harness contract) ----
BS = 524288
A = 51
N_CORES = 8
ROWS = BS // N_CORES          # 65536 rows per core
P = 128                       # partitions
G = 8                         # tiles per macro-tile
TILE_ROWS = P                 # rows per tile
MACRO_ROWS = P * G            # 1024
M = ROWS // MACRO_ROWS        # 64 macros

F = 2                         # dst window base
CS_P = 31744.0                # PP fixed-point scale
CS_V = 496.0                  # PV fixed-point scale
RATIO = CS_P / CS_V           # 64.0
NE = 104                      # local_scatter dst elements
W53 = 53
VS = 56                       # stride per group in the v-col tile

f32 = mybir.dt.float32
i16 = mybir.dt.int16


# ---------------------------------------------------------------------------
# custom DVE ops
# ---------------------------------------------------------------------------
_OPS = {}


def _register_ops():
    if _OPS:
        return _OPS
    from concourse import dve_ops as dvo
    from concourse.dve_spec import (
        Spec, Src0, Src1, C0, C1, C2, scan, AluOp, select, Idx, Zero, maxx,
        minn, lower, eq,
    )
    from concourse.dve_ops import has_src1
    from concourse.dve_table_gen import DveOpSpec

    def reg(name, spec, subdim=False):
        for existing in dvo.OPS:
            if existing.name == name:
                _OPS[name] = existing
                return
        row = dvo._CUSTOM_DVE_ROW_BASE + len(dvo.OPS)
        assert row < 0x20, "custom DVE row overflow"
        dvo._SUB_OPCODE_FOR_NAME[name] = row
        shas = {}
        for ver in ("v3", "v4"):
            s = DveOpSpec(name=name, opcode=row, uops=lower(spec, ver=ver),
                          rd1_en=has_src1(spec))
            shas[ver] = s.sha(ver)
        op = dvo.DveOp(name, spec, subdim, uops_sha=shas)
        dvo.OPS.append(op)
        dvo.CUSTOM_DVE_SPECS[name] = spec
        _OPS[name] = op

    # PP-scan: out_i16[k] = trunc(cumsum(p * CS_P))
    def _pp_ref(in0, in1, s0, s1, imm2):
        r = np.cumsum(in0.astype(np.float32) * np.float32(imm2), axis=-1,
                      dtype=np.float32)
        return np.clip(np.rint(r), -32768, 32767)

    reg("CP_PPSCAN", Spec(body=scan(AluOp.ADD, Src0 * C2), reference=_pp_ref))

    # PV-scan: out_i16[k] = trunc(cumsum(p * clip(c0 + c1*a, 0, imm2)))
    # in1 = const row of atom indices a = 0..50 (fp32)
    def _pv_ref(in0, in1, s0, s1, imm2):
        pos = np.clip(s0 + s1 * in1.astype(np.float32), 0.0, np.float32(imm2))
        r = np.cumsum(in0.astype(np.float32) * pos, axis=-1, dtype=np.float32)
        return np.clip(np.rint(r), -32768, 32767)

    reg("CP_PVSCAN",
        Spec(body=scan(AluOp.ADD, Src0 * minn(maxx(C0 + C1 * Src1, Zero), C2)),
             reference=_pv_ref))

    # redirect with cap: vc = min(in0, imm2); vn = min(in1, imm2);
    # out = (vc == vn) ? Idx + c0 : vc     (int16 streams)
    def _rd_ref(in0, in1, s0, s1, imm2):
        vc = np.minimum(in0.astype(np.float32), np.float32(imm2))
        vn = np.minimum(in1.astype(np.float32), np.float32(imm2))
        idx = np.arange(in0.shape[-1], dtype=np.float32)
        return np.where(vc == vn, idx + s0, vc)

    _vc = minn(Src0, C2)
    reg("CP_REDIR2", Spec(body=select(eq(_vc, minn(Src1, C2)), Idx + C0, _vc),
                          reference=_rd_ref))

    # R-op: gP = scanmax(in0), gV = scanmax(in1); R = gP*Idx - gV*c0
    def _r_ref(in0, in1, s0, s1, imm2):
        gP = np.maximum.accumulate(in0.astype(np.float32), axis=-1)
        gV = np.maximum.accumulate(in1.astype(np.float32), axis=-1)
        idx = np.arange(in0.shape[-1], dtype=np.float32)
        return gP * idx - gV * s0

    reg("CP_ROP", Spec(body=scan(AluOp.MAX, Src0) * Idx
                       - scan(AluOp.MAX, Src1) * C0, reference=_r_ref))

    # final: out = (in0 - in1) * c0
    def _fin_ref(in0, in1, s0, s1, imm2):
        return (in0.astype(np.float32) - in1.astype(np.float32)) * s0

    reg("CP_FINAL", Spec(body=(Src0 - Src1) * C0, reference=_fin_ref))
    return _OPS


# ---------------------------------------------------------------------------
# program builder
# ---------------------------------------------------------------------------
def _build(n_macros=M):
    ops = _register_ops()
    nc = bacc.Bacc()
    nrows = n_macros * MACRO_ROWS

    reward_in = nc.dram_tensor("reward", [nrows, 1], f32, kind="ExternalInput")
    probs_in = nc.dram_tensor("probs", [nrows, A], f32, kind="ExternalInput")
    nd_in = nc.dram_tensor("not_done", [nrows, 1], f32, kind="ExternalInput")
    crow_in = nc.dram_tensor("crow", [P, W53], f32, kind="ExternalInput")
    out_t = nc.dram_tensor("out", [nrows, A], f32, kind="ExternalOutput")

    # row(m, p, g) = m*1024 + p*8 + g
    pr = probs_in[:].rearrange("(m p g) c -> m p (g c)", m=n_macros, p=P, g=G)
    rr = reward_in[:].rearrange("(m p g) c -> m p (g c)", m=n_macros, p=P, g=G)
    ndr = nd_in[:].rearrange("(m p g) c -> m p (g c)", m=n_macros, p=P, g=G)
    outr = out_t[:].rearrange("(m p g) c -> m p (g c)", m=n_macros, p=P, g=G)

    AluOp = mybir.AluOpType

    with tile.TileContext(nc) as tc:
        with tc.tile_pool(name="consts", bufs=1) as cpool, \
             tc.tile_pool(name="work", bufs=2) as pool, \
             tc.tile_pool(name="dsts", bufs=2) as dpool:
            crow = cpool.tile([P, W53], f32)
            nc.sync.dma_start(out=crow[:], in_=crow_in[:])

            for mi in range(n_macros):
                ptile = pool.tile([P, G * A], f32, tag="ptile")
                nc.sync.dma_start(out=ptile[:], in_=pr[mi])
                rt = pool.tile([P, G], f32, tag="rt")
                nc.sync.dma_start(out=rt[:], in_=rr[mi])
                ndt = pool.tile([P, G], f32, tag="ndt")
                nc.sync.dma_start(out=ndt[:], in_=ndr[mi])

                # ---- scalar block (per-row values, one column per tile g)
                ACTF = mybir.ActivationFunctionType
                alphap = pool.tile([P, G], f32, tag="alphap")
                # alphap = 2.5*r + 25.5  (= alpha + 0.5)
                nc.vector.tensor_scalar(alphap[:], rt[:], 2.5, 25.5,
                                        AluOp.mult, AluOp.add)
                nc.vector.scalar_tensor_tensor(alphap[:], ndt[:], -24.75,
                                               alphap[:], AluOp.mult, AluOp.add)
                beta = pool.tile([P, G], f32, tag="beta")
                nc.scalar.activation(beta[:], ndt[:], ACTF.Copy, bias=0.0,
                                     scale=0.99)
                c0v = pool.tile([P, G], f32, tag="c0v")
                # c0v = alphap - beta = alpha - beta + 0.5
                nc.vector.scalar_tensor_tensor(c0v[:], beta[:], -1.0, alphap[:],
                                               AluOp.mult, AluOp.add)
                apv = pool.tile([P, G], f32, tag="apv")
                nc.scalar.activation(apv[:], alphap[:], ACTF.Copy,
                                     bias=-0.5 * CS_V, scale=CS_V)
                bpv = pool.tile([P, G], f32, tag="bpv")
                nc.scalar.activation(bpv[:], ndt[:], ACTF.Copy, bias=0.0,
                                     scale=0.99 * CS_V)

                # ---- per-macro tiles
                pps = pool.tile([P, G * 52], i16, tag="pps")
                pvs = pool.tile([P, G * 52], i16, tag="pvs")
                vcols = pool.tile([P, G * VS], i16, tag="vcols")
                cols = pool.tile([P, G * 52], i16, tag="cols")
                rtile = pool.tile([P, G * W53], f32, tag="rtile")
                dstp = dpool.tile([P, G * NE], i16, tag="dstp")
                dstv = dpool.tile([P, G * NE], i16, tag="dstv")

                pps_r = pps[:].rearrange("p (g c) -> p g c", g=G)
                pvs_r = pvs[:].rearrange("p (g c) -> p g c", g=G)
                rtile_r = rtile[:].rearrange("p (g c) -> p g c", g=G)

                # zero the per-group col-0 of the prefix tiles and R tiles,
                # and set the v-col sentinel column (ScalarE, off DVE)
                nd1 = ndt[:].rearrange("p (g o) -> p g o", o=1)
                vcols_r = vcols[:].rearrange("p (g c) -> p g c", g=G)
                nc.scalar.activation(pps_r[:, :, 0:1], nd1[:], ACTF.Copy,
                                     bias=0.0, scale=0.0)
                nc.scalar.activation(pvs_r[:, :, 0:1], nd1[:], ACTF.Copy,
                                     bias=0.0, scale=0.0)
                nc.scalar.activation(rtile_r[:, :, 0:1], nd1[:], ACTF.Copy,
                                     bias=0.0, scale=0.0)
                nc.scalar.activation(vcols_r[:, :, 52:53], nd1[:], ACTF.Copy,
                                     bias=-100.0, scale=0.0)

                for g in range(G):
                    psl = ptile[:, g * A:(g + 1) * A]
                    # prefix sums (int16 fixed point), exclusive via col 0 = 0
                    nc.vector._custom_dve(
                        ops["CP_PPSCAN"],
                        out=pps[:, g * 52 + 1:(g + 1) * 52],
                        in0=psl, imm2=CS_P)
                    nc.vector._custom_dve(
                        ops["CP_PVSCAN"],
                        out=pvs[:, g * 52 + 1:(g + 1) * 52],
                        in0=psl, in1=crow[:, 0:A],
                        s0=apv[:, g:g + 1], s1=bpv[:, g:g + 1],
                        imm2=50.0 * CS_V)
                    # scatter columns: col = round(relu(beta*c + c0v)) on ACT
                    nc.scalar.activation(
                        vcols[:, g * VS:g * VS + 52], crow[:, 0:52],
                        ACTF.Relu, bias=c0v[:, g:g + 1],
                        scale=beta[:, g:g + 1])
                    nc.vector._custom_dve(
                        ops["CP_REDIR2"],
                        out=cols[:, g * 52:(g + 1) * 52],
                        in0=vcols[:, g * VS:g * VS + 52],
                        in1=vcols[:, g * VS + 1:g * VS + W53],
                        s0=52.0, imm2=51.0)
                    # per-partition scatters
                    nc.gpsimd.local_scatter(
                        dstp[:, g * NE:(g + 1) * NE],
                        pps[:, g * 52:(g + 1) * 52],
                        cols[:, g * 52:(g + 1) * 52],
                        channels=P, num_elems=NE, num_idxs=52)
                    nc.gpsimd.local_scatter(
                        dstv[:, g * NE:(g + 1) * NE],
                        pvs[:, g * 52:(g + 1) * 52],
                        cols[:, g * 52:(g + 1) * 52],
                        channels=P, num_elems=NE, num_idxs=52)
                    # R[j] = scanmax-fill + j*gP - RATIO*gV  (PP units)
                    nc.vector._custom_dve(
                        ops["CP_ROP"],
                        out=rtile[:, g * W53 + 1:(g + 1) * W53],
                        in0=dstp[:, g * NE:g * NE + 52],
                        in1=dstv[:, g * NE:g * NE + 52],
                        s0=RATIO)

                # d = R[k+1] - R[k]; out = (d[k+1] - d[k]) / CS_P
                dtile = pool.tile([P, G * 52], f32, tag="dtile")
                dtile_r = dtile[:].rearrange("p (g c) -> p g c", g=G)
                nc.vector.tensor_tensor(
                    dtile_r[:], rtile_r[:, :, 1:53], rtile_r[:, :, 0:52],
                    AluOp.subtract)
                otile = pool.tile([P, G * A], f32, tag="otile")
                otile_r = otile[:].rearrange("p (g c) -> p g c", g=G)
                nc.vector._custom_dve(
                    ops["CP_FINAL"],
                    out=otile_r[:],
                    in0=dtile_r[:, :, 1:52], in1=dtile_r[:, :, 0:51],
                    s0=1.0 / CS_P)
                nc.scalar.dma_start(out=outr[mi], in_=otile[:])

    nc.compile()
    return nc


_CONSTS = None


def _const_inputs():
    global _CONSTS
    if _CONSTS is None:
        crow = np.tile(np.arange(W53, dtype=np.float32), (P, 1))
        _CONSTS = {"crow": crow}
    return _CONSTS


_PROGRAM = None


def kernel(reward, probs, not_done):
    global _PROGRAM
    reward = np.ascontiguousarray(np.asarray(reward, dtype=np.float32))
    probs = np.ascontiguousarray(np.asarray(probs, dtype=np.float32))
    not_done = np.ascontiguousarray(np.asarray(not_done, dtype=np.float32))
    assert reward.shape == (BS, 1) and probs.shape == (BS, A)

    if _PROGRAM is None:
        _PROGRAM = _build(M)
    consts = _const_inputs()

    in_maps = []
    for c in range(N_CORES):
        sl = slice(c * ROWS, (c + 1) * ROWS)
        in_maps.append({
            "reward": reward[sl],
            "probs": probs[sl],
            "not_done": not_done[sl],
            **consts,
        })
    res = run_bass_kernel_spmd(_PROGRAM, in_maps, list(range(N_CORES)))
    out = np.empty((BS, A), dtype=np.float32)
    for c in range(N_CORES):
        out[c * ROWS:(c + 1) * ROWS] = res.results[c]["out"]
    return out
